# revision 31
# baseline (speedup 1.0000x reference)
"""Multi-head attention (B=2, S=2048, D=1024, H=16, d_k=64) on 8 NeuronCores.

Sharding: data-parallel over batch (4 cores per batch element) x tensor-parallel
over heads (4 heads per core).  Each core computes its 256-wide slice of the
Q/K/V projections, attention for its 4 heads, and a partial output projection
(contribution of its head slice to all 1024 output dims).  Host sums the 4
partials per batch element and adds b_O.

Matmuls run in bf16 (fp32 accumulation in PSUM); softmax runs in fp32 on the
scalar engine (exp with the 1/sqrt(d_k) scale folded into the activation's
affine pre-scale).  The softmax denominator comes for free from 64 ones
columns appended to each head's V stationary (even heads [v|ones], odd heads
[ones|v]), so the PV matmul fills half its PSUM tile with the denominator
replicated across 64 partitions.  Normalization is then just a reciprocal and
a multiply at full DVE width, with no partition broadcast and no staging DMA:
the odd-head parity swap puts each head's attention rows at the partition
offset where attnT wants them.

All input DMA runs on the sync HWDGE ring in need-order, with x-tiles split
into column halves and interleaved with their weight tiles so the projections
start as soon as the first k-tile lands and track the DMA stream.  The ACT
exp stream paces the kernel; projections and out-projection fill PE gaps.
"""

import sys

sys.path.insert(0, "/opt/trn_rl_repo")

import numpy as np
import ml_dtypes

import concourse.bass as bass  # noqa: F401  (registers types)
import concourse.bacc as bacc
import concourse.mybir as mybir
import concourse.tile as tile
from concourse import bass_utils

BF16 = ml_dtypes.bfloat16

B = 2
S = 2048
D = 1024
N_HEAD = 16
DK = 64
HPC = 4            # heads per core
DPC = HPC * DK     # 256: per-core projection width
VW = 2 * DK        # v tile width per head (64 dims + 64 ones columns)
SC = 1024          # query-chunk (columns processed per attention pass)
NKT = S // 128     # 16 key tiles
NST = S // 128     # 16 sequence tiles
KD = D // 128      # 8 contraction tiles over D
N_CORES = 8
SCALE = 1.0 / np.sqrt(DK)

# smalls layout (f32, [128, 260]):
#   col 0/1: b_Q slice as two per-partition bias tiles
#   col 2/3: b_K slice
#   col 4..259: b_V as [128, 4, 64] (per head h: partitions x dims)
SM_BQ = 0
SM_BK = 2
SM_BV = 4
SM_W = 260

_cached_nc = None


def _build(dbg=False):
    dt = mybir.dt
    f32, bf16 = dt.float32, dt.bfloat16
    AF = mybir.ActivationFunctionType
    ALU = mybir.AluOpType

    nc = bacc.Bacc("TRN2", target_bir_lowering=False, debug=False,
                   num_devices=N_CORES)
    dbg_d = {}
    if dbg:
        for nm, shp in [("dv0", [128, HPC * VW]), ("dv1", [128, HPC * VW]),
                        ("dattn0", [128, S]), ("dattn1", [128, S])]:
            dbg_d[nm] = nc.dram_tensor(nm, shp, bf16, kind="ExternalOutput")

    xq_d = nc.dram_tensor("xq", [D, S], bf16, kind="ExternalInput")
    xk_d = nc.dram_tensor("xk", [D, S], bf16, kind="ExternalInput")
    xv_d = nc.dram_tensor("xv", [D, S], bf16, kind="ExternalInput")
    wq_d = nc.dram_tensor("wq", [D, DPC], bf16, kind="ExternalInput")
    wk_d = nc.dram_tensor("wk", [D, DPC], bf16, kind="ExternalInput")
    wv_d = nc.dram_tensor("wv", [D, DPC], bf16, kind="ExternalInput")
    wo_d = nc.dram_tensor("wo", [DPC, D], bf16, kind="ExternalInput")
    sm_d = nc.dram_tensor("smalls", [128, SM_W], f32, kind="ExternalInput")
    pout_d = nc.dram_tensor("pout", [S, D], f32, kind="ExternalOutput")

    with tile.TileContext(nc) as tc:
        with (
            tc.tile_pool(name="sb", bufs=1) as sb,
            tc.tile_pool(name="pts", bufs=10) as pts,
            tc.tile_pool(name="evs", bufs=4) as evs,
            tc.tile_pool(name="rps", bufs=1) as rps,
            tc.tile_pool(name="ppA", bufs=2, space="PSUM") as ppA,
            tc.tile_pool(name="ppB", bufs=1, space="PSUM") as ppB,
            tc.tile_pool(name="ppC", bufs=1, space="PSUM") as ppC,
        ):
            smalls = sb.tile([128, SM_W], f32, tag="smalls", name="smalls")
            nc.sync.dma_start(smalls[:], sm_d[:])

            def alloc_rows(n_tiles, width, tagbase):
                return [sb.tile([128, width], bf16, tag=f"{tagbase}{i}",
                                name=f"{tagbase}{i}")
                        for i in range(n_tiles)]

            def load_tile(t, dram, i, cols=None):
                if cols is None:
                    nc.sync.dma_start(t[:], dram[i * 128:(i + 1) * 128, :])
                else:
                    nc.sync.dma_start(
                        t[:, cols], dram[i * 128:(i + 1) * 128, cols])

            wk_t = alloc_rows(KD, DPC, "wk")
            xk_t = alloc_rows(KD, S, "xk")
            wq_t = alloc_rows(KD, DPC, "wq")
            xq_t = alloc_rows(KD, S, "xq")
            wv_t = alloc_rows(KD, DPC, "wv")
            xv_t = alloc_rows(KD, S, "xv")
            wo_t = alloc_rows(2, D, "wo")

            c0 = slice(0, SC)
            c1 = slice(SC, S)
            # input DMA in need-order on the sync HWDGE ring (strict FIFO);
            # x k-tiles interleaved with their weight tiles so projection
            # k-loop i can fire as soon as pair i lands.
            for i in range(KD):
                load_tile(xk_t[i], xk_d, i, c0)
                load_tile(wk_t[i], wk_d, i)
            for i in range(KD):
                load_tile(xq_t[i], xq_d, i, c0)
                load_tile(wq_t[i], wq_d, i)
            for i in range(KD):
                load_tile(xk_t[i], xk_d, i, c1)
                load_tile(xv_t[i], xv_d, i, c0)
                load_tile(wv_t[i], wv_d, i)
            for i in range(KD):
                load_tile(xq_t[i], xq_d, i, c1)
            for i in range(KD):
                load_tile(xv_t[i], xv_d, i, c1)
            for i in range(2):
                load_tile(wo_t[i], wo_d, i)

            # kTz[r][p][c]: rows [64p, 64p+64) hold head (2r+p)'s k.T for key
            # chunk c, the other 64 rows are zero.  QK uses these zero-padded
            # stationary tiles with the full 128-partition qT as moving
            # operand — the zero rows annihilate the other head's
            # contribution, keeping every matmul in plain 128x128 array mode
            # (no tiling-mode switches, which cost a PE drain each way).
            kTz = [[[sb.tile([128, SC], bf16, tag=f"kTz{r}{p}{c}",
                             name=f"kTz{r}{p}{c}") for c in range(2)]
                    for p in range(2)] for r in range(2)]
            for r in range(2):
                for c in range(2):
                    nc.gpsimd.memset(kTz[r][0][c][64:128, :], 0.0)
                    nc.gpsimd.memset(kTz[r][1][c][0:64, :], 0.0)
            qT = [[sb.tile([128, SC], bf16, tag=f"qT{r}{c}", name=f"qT{r}{c}")
                   for c in range(2)] for r in range(2)]
            v_t = [sb.tile([128, HPC * VW], bf16, tag=f"v{i}", name=f"v{i}")
                   for i in range(NST)]
            attnT = [sb.tile([128, S], bf16, tag=f"attnT{r}", name=f"attnT{r}")
                     for r in range(2)]

            # ---- K / Q projections: dst.T[j, s] = sum_d W[d, j] * X[d, s] ----
            def gen_proj_qk(w_tiles, x_tiles, dst, bias_col, m, n0, pool,
                            ptag):
                ps = pool.tile([128, SC], f32, tag=ptag,
                               name=f"psp{bias_col}{m}{n0}")
                for k in range(KD):
                    for h2 in range(2):
                        cc = n0 * SC + h2 * 512
                        nc.tensor.matmul(
                            ps[:, h2 * 512:(h2 + 1) * 512],
                            lhsT=w_tiles[k][:, m * 128:(m + 1) * 128],
                            rhs=x_tiles[k][:, cc:cc + 512],
                            start=(k == 0), stop=(k == KD - 1))
                        yield
                if dst is None:  # K projection into zero-padded kTz tiles
                    for p in range(2):
                        pr = slice(p * DK, (p + 1) * DK)
                        nc.vector.tensor_scalar_add(
                            kTz[m][p][n0][pr, :], ps[pr, :],
                            smalls[pr, bias_col + m:bias_col + m + 1])
                else:
                    nc.vector.tensor_scalar_add(
                        dst[m][n0][:, :], ps[:, :],
                        smalls[:, bias_col + m:bias_col + m + 1])

            def proj_qk_chunk(*args):
                for _ in gen_proj_qk(*args):
                    pass

            def make_filler(gens, steps_per_call):
                state = list(gens)

                def filler(kt):
                    n = steps_per_call
                    while n > 0 and state:
                        try:
                            next(state[0])
                            n -= 1
                        except StopIteration:
                            state.pop(0)

                def drain():
                    while state:
                        try:
                            next(state[0])
                        except StopIteration:
                            state.pop(0)

                filler.drain = drain
                return filler

            def make_filler_rr(quota_gens):
                # round-robin filler: each call takes quota_i steps from each
                # generator in order (first listed runs first every call)
                state = [[g, q] for g, q in quota_gens]

                def filler(kt):
                    for ent in list(state):
                        n = ent[1]
                        while n > 0:
                            try:
                                next(ent[0])
                                n -= 1
                            except StopIteration:
                                state.remove(ent)
                                break

                def drain():
                    for ent in list(state):
                        for _ in ent[0]:
                            pass
                    state.clear()

                filler.drain = drain
                return filler

            bvv = smalls[:, SM_BV:SM_BV + HPC * DK].rearrange(
                "p (h x) -> p h x", x=DK)

            def gen_proj_v():
                # v_aug per head h: [v | 64 ones columns] so PV puts the
                # attention rows at partitions 0-63 and the softmax
                # denominator replicated across partitions 64-127.
                for st in range(NST):
                    pv = ppC.tile([128, DPC], f32, tag="C", name=f"pv{st}")
                    for k in range(KD):
                        nc.tensor.matmul(
                            pv[:, :],
                            lhsT=xv_t[k][:, st * 128:(st + 1) * 128],
                            rhs=wv_t[k][:, :],
                            start=(k == 0), stop=(k == KD - 1))
                        yield
                    vv = v_t[st][:].rearrange("p (h x) -> p h x", x=VW)
                    pvv = pv[:].rearrange("p (h e) -> p h e", e=DK)
                    nc.vector.tensor_tensor(vv[:, :, 0:DK], pvv, bvv,
                                            op=ALU.add)
                    nc.vector.memset(vv[:, :, DK:VW], 1.0)
                    yield

            # The attention phase is ACT(exp)-paced: the QK+exp stream leads
            # the PV stream by PIPE kt positions (across head boundaries), so
            # the ACT exp pipeline never drains while a head's trailing PV /
            # normalize chain completes.
            PIPE = 3

            def emit_qk(heads, p, pt_q):
                hi, kt = divmod(p, NKT)
                n0, h = heads[hi]
                r = h // 2
                if p <= 8:
                    qk_fill(p)
                ps = ppA.tile([128, SC], f32, tag="A", name=f"ps{n0}{h}{kt}")
                for h2 in range(2):
                    nc.tensor.matmul(
                        ps[:, h2 * 512:(h2 + 1) * 512],
                        lhsT=kTz[r][h % 2][kt // 8][
                            :, (kt % 8) * 128:(kt % 8 + 1) * 128],
                        rhs=qT[r][n0][:, h2 * 512:(h2 + 1) * 512],
                        start=True, stop=True)
                pt = pts.tile([128, SC], bf16, tag="pt", name=f"pt{n0}{h}{kt}")
                nc.scalar.activation(pt[:], ps[:], AF.Exp, scale=float(SCALE))
                pt_q[p] = pt

            def normalize(n0, h, pa):
                q0 = n0 * SC
                r, off = h // 2, (h % 2) * DK
                dn = rps.tile([DK, SC], f32, tag="dn", name=f"dn{n0}{h}")
                rb = rps.tile([DK, SC], f32, tag="rb", name=f"rb{n0}{h}")
                nc.vector.tensor_copy(dn[:, :], pa[DK:128, :])
                nc.vector.reciprocal_approx_fast(rb[:, :], dn[:, :])
                if off == 0:
                    for hh in range(2):
                        cs = slice(hh * 512, (hh + 1) * 512)
                        nc.vector.tensor_tensor(
                            attnT[r][0:DK, q0 + hh * 512:q0 + (hh + 1) * 512],
                            pa[0:DK, cs], rb[:, cs], op=ALU.mult)
                else:
                    stg = rps.tile([DK, SC], bf16, tag="stg",
                                   name=f"stg{n0}{h}")
                    nc.vector.tensor_tensor(stg[:, :], pa[0:DK, :], rb[:, :],
                                            op=ALU.mult)
                    nc.gpsimd.dma_start(
                        attnT[r][off:off + DK, q0:q0 + SC], stg[:, :])

            def attn_pipeline(heads, fillers):
                total = len(heads) * NKT
                pt_q = {}
                pa_cur = [None]

                def emit_pv(p):
                    hi, kt = divmod(p, NKT)
                    n0, h = heads[hi]
                    if kt == 0:
                        pa_cur[0] = ppB.tile([128, SC], f32, tag="B",
                                             name=f"pa{n0}{h}")
                    f = fillers[hi]
                    if f is not None:
                        f(kt)
                    pa = pa_cur[0]
                    pt = pt_q.pop(p)
                    for h2 in range(2):
                        nc.tensor.matmul(
                            pa[:, h2 * 512:(h2 + 1) * 512],
                            lhsT=v_t[kt][:, h * VW:(h + 1) * VW],
                            rhs=pt[:, h2 * 512:(h2 + 1) * 512],
                            start=(kt == 0), stop=(kt == NKT - 1))
                    if kt == NKT - 1:
                        normalize(n0, h, pa)
                        if f is not None:
                            f.drain()

                for p in range(total + PIPE):
                    if p < total:
                        emit_qk(heads, p, pt_q)
                    if p >= PIPE:
                        emit_pv(p - PIPE)

            def gen_outproj(sts, pool, ptag, use_act):
                for i, st in enumerate(sts):
                    for h2 in range(2):
                        po = pool.tile([128, 512], f32, tag=ptag,
                                       name=f"po{st}{h2}")
                        for jt in range(2):
                            nc.tensor.matmul(
                                po[:, :],
                                lhsT=attnT[jt][:, st * 128:(st + 1) * 128],
                                rhs=wo_t[jt][:, h2 * 512:(h2 + 1) * 512],
                                start=(jt == 0), stop=(jt == 1))
                            yield
                        og = evs.tile([128, 512], f32, tag="og",
                                      name=f"og{st}{h2}")
                        if use_act and h2 == 0:
                            nc.scalar.copy(og[:], po[:])
                        else:
                            nc.vector.tensor_copy(og[:], po[:])
                        nc.sync.dma_start(
                            pout_d[st * 128:(st + 1) * 128,
                                   h2 * 512:(h2 + 1) * 512], og[:])
                        yield

            def interleave(*gens):
                gens = list(gens)
                while gens:
                    g = gens.pop(0)
                    try:
                        next(g)
                        gens.append(g)
                    except StopIteration:
                        pass

            # Emission order = scheduling priority.  Attention heads feed the
            # ACT exp stream; remaining projection / out-projection work is
            # smeared into the attention kt-loops as fine-grained PE filler.
            # Head order 0,1,3,2: each chunk ends on an even head (direct
            # attnT write at partition 0) so the out-projection's last
            # dependency is produced with the shortest normalize chain.
            proj_qk_chunk(wk_t, xk_t, None, SM_BK, 0, 0, ppA, "A")
            proj_qk_chunk(wq_t, xq_t, qT, SM_BQ, 0, 0, ppA, "A")

            heads = [(0, 0), (0, 1), (0, 3), (0, 2),
                     (1, 0), (1, 1), (1, 3), (1, 2)]
            # K chunk-1 projection is fed into the first head's QK stream (2
            # matmuls per kt): as a prologue chunk it would head the PE FIFO
            # while waiting on its DMA and block all attention behind it.
            qk_fill = make_filler(
                [gen_proj_qk(wk_t, xk_t, None, SM_BK, 0, 1, ppB, "B")], 2)
            fillers = [
                make_filler([gen_proj_v()], 11),
                make_filler([
                    gen_proj_qk(wk_t, xk_t, None, SM_BK, 1, 0, ppC, "C"),
                    gen_proj_qk(wk_t, xk_t, None, SM_BK, 1, 1, ppC, "C"),
                    gen_proj_qk(wq_t, xq_t, qT, SM_BQ, 1, 0, ppC, "C"),
                ], 6),
                make_filler([gen_proj_qk(wq_t, xq_t, qT, SM_BQ, 0, 1,
                                         ppC, "C")], 2),
                make_filler([], 0),
                make_filler([gen_proj_qk(wq_t, xq_t, qT, SM_BQ, 1, 1,
                                         ppC, "C"),
                             gen_outproj((0, 1), ppC, "C", False)], 3),
                make_filler([gen_outproj((2, 3), ppC, "C", False)], 2),
                make_filler([gen_outproj((4, 5), ppC, "C", False),
                             gen_outproj((6, 7), ppC, "C", False)], 4),
                make_filler([], 0),
            ]
            attn_pipeline(heads, fillers)
            # tail out-projection: four chains on separate PSUM slots so the
            # po->og->DMA pipelines overlap instead of serializing on slots
            interleave(gen_outproj((8, 12), ppA, "A", True),
                       gen_outproj((9, 13), ppB, "B", False),
                       gen_outproj((10, 14), ppC, "C", True),
                       gen_outproj((11, 15), ppA, "A", False))
            if dbg:
                nc.sync.dma_start(dbg_d["dv0"][:], v_t[0][:])
                nc.sync.dma_start(dbg_d["dv1"][:], v_t[1][:])
                nc.sync.dma_start(dbg_d["dattn0"][:], attnT[0][:])
                nc.sync.dma_start(dbg_d["dattn1"][:], attnT[1][:])

    nc.compile()
    return nc


def _get_nc():
    global _cached_nc
    if _cached_nc is None:
        _cached_nc = _build()
    return _cached_nc


def _make_in_maps(Q, K, V, W_Q, b_Q, W_K, b_K, W_V, b_V, W_O, b_O):
    in_maps = []
    for c in range(N_CORES):
        b, g = c // 4, c % 4
        hs = slice(g * DPC, (g + 1) * DPC)
        smalls = np.zeros((128, SM_W), np.float32)
        smalls[:, SM_BQ] = b_Q[hs][:128]
        smalls[:, SM_BQ + 1] = b_Q[hs][128:]
        smalls[:, SM_BK] = b_K[hs][:128]
        smalls[:, SM_BK + 1] = b_K[hs][128:]
        smalls[:, SM_BV:SM_BV + HPC * DK] = b_V[hs].reshape(-1)[None, :]
        in_maps.append({
            "xq": np.ascontiguousarray(Q[b].T).astype(BF16),
            "xk": np.ascontiguousarray(K[b].T).astype(BF16),
            "xv": np.ascontiguousarray(V[b].T).astype(BF16),
            "wq": np.ascontiguousarray(W_Q[hs, :].T).astype(BF16),
            "wk": np.ascontiguousarray(W_K[hs, :].T).astype(BF16),
            "wv": np.ascontiguousarray(W_V[hs, :].T).astype(BF16),
            "wo": np.ascontiguousarray(W_O[:, hs].T).astype(BF16),
            "smalls": smalls,
        })
    return in_maps


def _gather(results, b_O):
    out = np.zeros((B, S, D), np.float32)
    for c in range(N_CORES):
        out[c // 4] += results[c]["pout"]
    out += b_O[None, None, :]
    return out


def run(trace=False, **inputs):
    nc = _get_nc()
    in_maps = _make_in_maps(**inputs)
    res = bass_utils.run_bass_kernel_spmd(
        nc, in_maps, core_ids=list(range(N_CORES)), trace=trace)
    return _gather(res.results, np.asarray(inputs["b_O"], np.float32)), res


def kernel(**inputs):
    out, _ = run(trace=False, **inputs)
    return out
